# revision 12
# baseline (speedup 1.0000x reference)
"""Swin-style window attention kernel for Trainium2 (8 NeuronCores, data-parallel).

Computes, for x:[2048,49,384]:
    qkv = x @ qkv_w.T + qkv_b ; split into q,k,v heads (12 x 32)
    attn = softmax(q k^T / sqrt(32) + rel_pos_bias + window_mask)
    out  = (attn @ v) @ proj_w.T

Strategy: data-parallel over the leading B_ axis (256 windows / core).
On-chip layout is channel-major (x pre-transposed on host), windows are
processed in pairs (98 tokens) so attention matmuls use 98-wide tiles.
Relative-position bias + window mask are folded into one multiplicative
term EB = exp(bias + mask) precomputed on the host; softmax is computed
without max-subtraction (scores are O(1) here) as exp(s)*EB / colsum.
All matmuls run in bf16 with fp32 PSUM accumulation.

v3: software-pipelined across blocks — at step s the PE stream interleaves
qk/v GEMM chunks for block s, attention pairs for block s-1, and proj
chunks for block s-2, so the PE has GEMM work to execute while ScalarE
runs exp and the DVE runs the EB/normalize multiplies.  PSUM evacuation
is split between ScalarE and VectorE; part of the EB multiply goes to
GPSIMD.
"""

import sys

sys.path.insert(0, "/opt/trn_rl_repo")

import numpy as np
import ml_dtypes

import concourse.bacc as bacc
import concourse.mybir as mybir
import concourse.tile as tile
from concourse.bass_utils import run_bass_kernel_spmd

BF16 = ml_dtypes.bfloat16
F32 = np.float32

N_CORES = 8
D, H, HD = 384, 12, 32
WN = 49                      # tokens per window
NW = 64                      # distinct window masks
B_ = 2048
B_CORE = B_ // N_CORES       # 256 windows per core
T_CORE = B_CORE * WN         # 12544 tokens per core
PT = 2 * WN                  # 98 tokens per window pair
N_PAIR = B_CORE // 2         # 128 pairs per core
PAIR_PAT = NW // 2           # 32 distinct pair mask patterns
BLK_PAIRS = 8
BLK_T = BLK_PAIRS * PT       # 784 tokens per block
N_BLK = N_PAIR // BLK_PAIRS  # 16 blocks per core
NH = BLK_T // 2              # 392: half-block free dim for 512-limit psum
SCALE = HD ** (-0.5)

_BF = mybir.dt.bfloat16
_F32 = mybir.dt.float32

# which pairs' EB-multiply runs on GPSIMD (rest on DVE)
GPS_EB_PAIRS = frozenset({2, 5})


def _relative_position_index():
    coords = np.stack(np.meshgrid(np.arange(7), np.arange(7), indexing="ij"))
    cf = coords.reshape(2, -1)
    rel = cf[:, :, None] - cf[:, None, :]
    rel = rel.transpose(1, 2, 0).copy()
    rel[:, :, 0] += 6
    rel[:, :, 1] += 6
    rel[:, :, 0] *= 13
    return rel.sum(-1)  # [49, 49] int


def _chunk_range(p8: int, n: int):
    return range(p8 * n // BLK_PAIRS, (p8 + 1) * n // BLK_PAIRS)


def _build_nc(qkv_bias_nonzero: bool, reps: int = 1, skip: frozenset = frozenset()):
    nc = bacc.Bacc("TRN2", target_bir_lowering=False, debug=True)

    xT_d = nc.dram_tensor("xT", [D, T_CORE], _BF, kind="ExternalInput")
    wqk_d = nc.dram_tensor("wqk", [128, 3, 2 * D], _BF, kind="ExternalInput")
    wv_d = nc.dram_tensor("wv", [128, 3, D], _BF, kind="ExternalInput")
    pw_d = nc.dram_tensor("pw", [128, 3, D], _BF, kind="ExternalInput")
    eb_d = nc.dram_tensor("eb", [PT, PAIR_PAT, H, PT], _BF, kind="ExternalInput")
    bqk_d = nc.dram_tensor("bqk", [1, 2 * D], _BF, kind="ExternalInput")
    bv_d = nc.dram_tensor("bv", [1, D], _BF, kind="ExternalInput")
    yT_d = nc.dram_tensor("yT", [D, T_CORE], _BF, kind="ExternalOutput")

    xT_view = xT_d[:, :].rearrange("(k p) t -> p k t", p=128)
    yT_view = yT_d[:, :].rearrange("(k p) t -> p k t", p=128)

    TOT = N_BLK * reps

    with tile.TileContext(nc) as tc:
        with (
            tc.tile_pool(name="consts", bufs=1) as consts,
            tc.tile_pool(name="xin", bufs=2) as xin,
            tc.tile_pool(name="qkp", bufs=2) as qkp,
            tc.tile_pool(name="vp", bufs=2) as vp,
            tc.tile_pool(name="attnp", bufs=4) as attnp,
            tc.tile_pool(name="rp", bufs=2) as rp,
            tc.tile_pool(name="outp", bufs=3) as outp,
            tc.tile_pool(name="yp", bufs=2) as yp,
            tc.tile_pool(name="ps_mm", bufs=2, space="PSUM") as ps_mm,
            tc.tile_pool(name="ps_s", bufs=1, space="PSUM") as ps_s,
            tc.tile_pool(name="ps_cs", bufs=1, space="PSUM") as ps_cs,
            tc.tile_pool(name="ps_o", bufs=1, space="PSUM") as ps_o,
        ):
            # ---- constants ----
            wqk_sb = consts.tile([128, 3, 2 * D], _BF)
            nc.sync.dma_start(out=wqk_sb, in_=wqk_d[:, :, :])
            wv_sb = consts.tile([128, 3, D], _BF)
            nc.sync.dma_start(out=wv_sb, in_=wv_d[:, :, :])
            pw_sb = consts.tile([128, 3, D], _BF)
            nc.sync.dma_start(out=pw_sb, in_=pw_d[:, :, :])
            eb_sb = consts.tile([PT, PAIR_PAT, H, PT], _BF)
            nc.sync.dma_start(out=eb_sb, in_=eb_d[:, :, :, :])
            ones_sb = consts.tile([PT, 32], _BF)
            nc.vector.memset(ones_sb, 1.0)
            if "scores" in skip:
                attn0_sb = consts.tile([PT, H, PT], _BF, name="attn0")
                nc.gpsimd.memset(attn0_sb, 0.5)
            if "av" in skip:
                outN0_sb = consts.tile([128, 3, BLK_PAIRS, PT], _BF, name="outN0")
                nc.gpsimd.memset(outN0_sb, 0.5)
            if qkv_bias_nonzero:
                bqk_sb = consts.tile([1, 2 * D], _BF)
                nc.sync.dma_start(out=bqk_sb, in_=bqk_d[:, :])
                bv_sb = consts.tile([1, D], _BF)
                nc.sync.dma_start(out=bv_sb, in_=bv_d[:, :])
                onetok_sb = consts.tile([1, NH], _BF)
                nc.vector.memset(onetok_sb, 1.0)

            # ---- per-block helpers ----
            def dma_x(s, dst):
                t0 = (s % N_BLK) * BLK_T
                nc.sync.dma_start(out=dst, in_=xT_view[:, :, t0 : t0 + BLK_T])

            def qk_chunk(xT_sb, qk_sb, c):
                nh, m = c // 6, c % 6
                mm_ps = ps_mm.tile([128, NH], _F32, tag="mm")
                for k in range(3):
                    nc.tensor.matmul(
                        out=mm_ps,
                        lhsT=wqk_sb[:, k, 128 * m : 128 * (m + 1)],
                        rhs=xT_sb[:, k, nh * NH : (nh + 1) * NH],
                        start=(k == 0),
                        stop=(k == 2) if not qkv_bias_nonzero else False,
                    )
                if qkv_bias_nonzero:
                    nc.tensor.matmul(
                        out=mm_ps,
                        lhsT=bqk_sb[:, 128 * m : 128 * (m + 1)],
                        rhs=onetok_sb,
                        start=False,
                        stop=True,
                    )
                dst = qk_sb[:, m, nh * NH : (nh + 1) * NH]
                if c not in (5, 11):
                    nc.scalar.copy(out=dst, in_=mm_ps)
                else:
                    nc.vector.tensor_copy(out=dst, in_=mm_ps)

            def v_chunk(xT_sb, v_sb, p8):
                v_ps = ps_mm.tile([PT, D], _F32, tag="mm", name="v_ps")
                for k in range(3):
                    nc.tensor.matmul(
                        out=v_ps,
                        lhsT=xT_sb[:, k, p8 * PT : (p8 + 1) * PT],
                        rhs=wv_sb[:, k, :],
                        start=(k == 0),
                        stop=(k == 2) if not qkv_bias_nonzero else False,
                    )
                if qkv_bias_nonzero:
                    nc.tensor.matmul(
                        out=v_ps,
                        lhsT=onetok_sb[:, :PT],
                        rhs=bv_sb,
                        start=False,
                        stop=True,
                    )
                if p8 % 3 == 0:
                    nc.scalar.copy(out=v_sb[:, p8, :], in_=v_ps)
                else:
                    nc.vector.tensor_copy(out=v_sb[:, p8, :], in_=v_ps)

            def scores_exp(qk_sb, blk, p8):
                """scores matmuls + exp + EB-mul for pair (blk, p8).
                Returns the attn SBUF tile."""
                pr = (blk * BLK_PAIRS + p8) % PAIR_PAT
                ts = p8 * PT
                if "scores" in skip:
                    return attn0_sb
                # one 4-bank PSUM tile; bank j <- row-group j only
                s_ps = ps_s.tile([PT, 16, 128], _F32, tag="s", name="s4")
                for r in range(3):
                    for j in range(4):
                        g = (j + r) % 3
                        nc.tensor.matmul(
                            out=s_ps[:, 4 * j + g, :PT],
                            lhsT=qk_sb[32 * j : 32 * (j + 1), 3 + g, ts : ts + PT],
                            rhs=qk_sb[32 * j : 32 * (j + 1), g, ts : ts + PT],
                            start=True,
                            stop=True,
                            tile_position=(32 * j, 0),
                        )
                # attn = exp(s) * EB; head order (j, g): idx = 3j+g = head 4g+j
                # single exp/EB: the 2-pair stagger provides the chain slack
                attn_sb = attnp.tile([PT, H, PT], _BF)
                s_v = s_ps.rearrange("p (j g) c -> p j g c", g=4)
                nc.scalar.activation(
                    out=attn_sb[:, :, :],
                    in_=s_v[:, :, :3, :PT],
                    func=mybir.ActivationFunctionType.Exp,
                )
                if "ebmul" not in skip:
                    if p8 in GPS_EB_PAIRS:
                        nc.gpsimd.tensor_mul(attn_sb, attn_sb, eb_sb[:, pr, :, :])
                    else:
                        nc.vector.tensor_mul(attn_sb, attn_sb, eb_sb[:, pr, :, :])
                return attn_sb

            def colsum_recip(attn_sb):
                cs_ps = ps_cs.tile([128, 3, PT], _F32, tag="cs")
                for r in range(4):
                    for g in range(3):
                        j = (g + r) % 4
                        nc.tensor.matmul(
                            out=cs_ps[32 * j : 32 * (j + 1), g, :],
                            lhsT=ones_sb,
                            rhs=attn_sb[:, 3 * j + g, :],
                            start=True,
                            stop=True,
                            tile_position=(0, 32 * j),
                        )
                r_sb = rp.tile([128, 3, PT], _F32, tag="rsb")
                nc.vector.reciprocal_approx_fast(out=r_sb, in_=cs_ps)
                return r_sb

            def av_norm(v_sb, attn_sb, r_sb, outN_sb, p8):
                o_ps = ps_o.tile([128, 3, PT], _F32, tag="o")
                for r in range(4):
                    for g in range(3):
                        j = (g + r) % 4
                        h = 4 * g + j
                        nc.tensor.matmul(
                            out=o_ps[32 * j : 32 * (j + 1), g, :],
                            lhsT=v_sb[:, p8, 32 * h : 32 * (h + 1)],
                            rhs=attn_sb[:, 3 * j + g, :],
                            start=True,
                            stop=True,
                            tile_position=(0, 32 * j),
                        )
                nc.vector.tensor_mul(
                    outN_sb[:, :, p8, :], o_ps[:, :, :], r_sb[:, :, :]
                )

            def proj_chunk(outN_sb, yT_sb, c):
                nh, m = c // 3, c % 3
                y_ps = ps_mm.tile([128, NH], _F32, tag="mm")
                for k in range(3):
                    nc.tensor.matmul(
                        out=y_ps,
                        lhsT=pw_sb[:, k, 128 * m : 128 * (m + 1)],
                        rhs=outN_sb[:, k, nh * 4 : (nh + 1) * 4, :],
                        start=(k == 0),
                        stop=(k == 2),
                    )
                dst = yT_sb[:, m, nh * NH : (nh + 1) * NH]
                if c % 2 == 0:
                    nc.scalar.copy(out=dst, in_=y_ps)
                else:
                    nc.vector.tensor_copy(out=dst, in_=y_ps)

            # ---- software-pipelined main loop ----
            # step s: qk/v for block s, attention pairs for block s-1,
            # proj for block s-2, DMA x for s+1, DMA y for s-2.
            x_cur = None      # xT tile for block s
            qk_prev = qk_cur = None
            v_prev = v_cur = None
            outN_cur = outN_prev = None   # outN for block s-1 / s-2
            pend = []         # (attn_sb, v_sb, outN_sb, p8) awaiting colsum/av

            x_cur = xin.tile([128, 3, BLK_T], _BF)
            dma_x(0, x_cur)

            for s in range(TOT + 2):
                do_qkv = s < TOT
                do_attn = 0 <= s - 1 < TOT
                do_proj = 0 <= s - 2 < TOT

                qk_prev, qk_cur = qk_cur, (
                    qkp.tile([128, 6, BLK_T], _BF, name="qk_sb")
                    if do_qkv else None
                )
                v_prev, v_cur = v_cur, (
                    vp.tile([PT, BLK_PAIRS, D], _BF, name="v_sb")
                    if do_qkv else None
                )
                if do_qkv:
                    if s + 1 < TOT:
                        x_next = xin.tile([128, 3, BLK_T], _BF)
                        dma_x(s + 1, x_next)
                    else:
                        x_next = None
                outN_prev, outN_cur = outN_cur, (
                    (outN0_sb if "av" in skip
                     else outp.tile([128, 3, BLK_PAIRS, PT], _BF, name="outN_sb"))
                    if do_attn else None
                )
                if do_proj:
                    yT_sb = yp.tile([128, 3, BLK_T], _BF)

                for p8 in range(BLK_PAIRS):
                    if do_qkv:
                        for c in _chunk_range(p8, 12):
                            qk_chunk(x_cur, qk_cur, c)
                        v_chunk(x_cur, v_cur, p8)

                    if do_attn:
                        blk = (s - 1) % N_BLK
                        attn_sb = scores_exp(qk_prev, blk, p8)
                        if "av" not in skip:
                            pend.append((attn_sb, v_prev, outN_cur, p8))

                    # colsum/av + normalize for the pair TWO slots back, so
                    # exp+EBmul get two pair-periods of slack before the PE
                    # needs the attn tile.
                    if len(pend) > 2:
                        p_attn, p_v, p_outN, p_p8 = pend.pop(0)
                        r_sb = colsum_recip(p_attn)
                        av_norm(p_v, p_attn, r_sb, p_outN, p_p8)

                    if do_proj:
                        for c in _chunk_range(p8, 6):
                            proj_chunk(outN_prev, yT_sb, c)

                if do_qkv:
                    x_cur = x_next
                if do_proj:
                    t0 = ((s - 2) % N_BLK) * BLK_T
                    nc.sync.dma_start(
                        out=yT_view[:, :, t0 : t0 + BLK_T], in_=yT_sb
                    )

                # drain the pending pairs of the final attention block
                if s - 1 == TOT - 1:
                    while pend:
                        p_attn, p_v, p_outN, p_p8 = pend.pop(0)
                        r_sb = colsum_recip(p_attn)
                        av_norm(p_v, p_attn, r_sb, p_outN, p_p8)

    nc.compile()
    return nc


_NC_CACHE: dict = {}


def _get_nc(qkv_bias_nonzero: bool):
    key = qkv_bias_nonzero
    if key not in _NC_CACHE:
        _NC_CACHE[key] = _build_nc(qkv_bias_nonzero)
    return _NC_CACHE[key]


def _host_prep(x, mask, qkv_w, qkv_b, proj_w, rpb_table):
    """Build per-core input maps (all device tensors bf16)."""
    # x^T per core: [384, 12544], channel-major
    x8 = np.ascontiguousarray(x, dtype=F32).reshape(N_CORES, T_CORE, D)

    # weights: lhsT layout [ci, co] chunked as [128, 3, co]
    wqkv_t = np.ascontiguousarray(qkv_w, dtype=F32).T  # [384, 1152] = [ci, co]
    wqk = wqkv_t[:, : 2 * D].copy()
    wqk[:, :D] *= SCALE  # fold 1/sqrt(hd) into q weights
    wv = wqkv_t[:, 2 * D :]
    pw_t = np.ascontiguousarray(proj_w, dtype=F32).T  # [ci, co]

    def chunk(w):  # [384, co] -> [128, 3, co]
        return np.ascontiguousarray(
            w.reshape(3, 128, w.shape[1]).transpose(1, 0, 2)
        ).astype(BF16)

    wqk_a, wv_a, pw_a = chunk(wqk), chunk(wv), chunk(pw_t)

    # EB = exp(biasT + maskT) per pair pattern, [98, 32, 12, 98] (j, pr, h, i)
    rpi = _relative_position_index()
    bias = np.asarray(rpb_table, dtype=F32)[rpi]          # [i, j, H]
    biasT = bias.transpose(2, 1, 0)                        # [H, j, i]
    maskT = np.asarray(mask, dtype=F32).transpose(0, 2, 1)  # [w, j, i]
    mb = np.full((PAIR_PAT, H, PT, PT), -30000.0, dtype=F32)
    mb[:, :, :WN, :WN] = biasT[None] + maskT[0::2, None, :, :]
    mb[:, :, WN:, WN:] = biasT[None] + maskT[1::2, None, :, :]
    eb = np.exp(mb)
    # device head order is (j, g): idx = 3j + g holds head h = 4g + j
    perm = np.array([4 * (i % 3) + i // 3 for i in range(H)])
    eb = eb[:, perm]
    eb_a = np.ascontiguousarray(eb.transpose(2, 0, 1, 3)).astype(BF16)

    b = np.asarray(qkv_b, dtype=F32)
    bqk = b[: 2 * D].copy()
    bqk[:D] *= SCALE
    bqk_a = bqk[None, :].astype(BF16)
    bv_a = b[2 * D :][None, :].astype(BF16)

    in_maps = []
    for c in range(N_CORES):
        xT_c = np.ascontiguousarray(x8[c].T).astype(BF16)  # [384, 12544]
        in_maps.append(
            {
                "xT": xT_c,
                "wqk": wqk_a,
                "wv": wv_a,
                "pw": pw_a,
                "eb": eb_a,
                "bqk": bqk_a,
                "bv": bv_a,
            }
        )
    return in_maps


def kernel(x, mask, qkv_w, qkv_b, proj_w, rpb_table, _want_trace=False):
    qkv_bias_nonzero = bool(np.any(np.asarray(qkv_b) != 0))
    nc = _get_nc(qkv_bias_nonzero)
    in_maps = _host_prep(x, mask, qkv_w, qkv_b, proj_w, rpb_table)
    res = run_bass_kernel_spmd(
        nc, in_maps, core_ids=list(range(N_CORES))
    )
    yT = np.stack([res.results[c]["yT"].astype(F32) for c in range(N_CORES)])
    y = yT.transpose(0, 2, 1).reshape(B_, WN, D)
    return y



# revision 15
# speedup vs baseline: 1.0611x; 1.0611x over previous
"""Swin-style window attention kernel for Trainium2 (8 NeuronCores, data-parallel).

Computes, for x:[2048,49,384]:
    qkv = x @ qkv_w.T + qkv_b ; split into q,k,v heads (12 x 32)
    attn = softmax(q k^T / sqrt(32) + rel_pos_bias + window_mask)
    out  = (attn @ v) @ proj_w.T

Strategy: data-parallel over the leading B_ axis (256 windows / core).
On-chip layout is channel-major (x pre-transposed on host), windows are
processed in pairs (98 tokens) so attention matmuls use 98-wide tiles.
Relative-position bias + window mask are folded into one multiplicative
term EB = exp(bias + mask) precomputed on the host; softmax is computed
without max-subtraction (scores are O(1) here) as exp(s)*EB / colsum.
All matmuls run in bf16 with fp32 PSUM accumulation.

v3: software-pipelined across blocks — at step s the PE stream interleaves
qk/v GEMM chunks for block s, attention pairs for block s-1, and proj
chunks for block s-2, so the PE has GEMM work to execute while ScalarE
runs exp and the DVE runs the EB/normalize multiplies.  PSUM evacuation
is split between ScalarE and VectorE; part of the EB multiply goes to
GPSIMD.

v10 (current): the colsum/av matmuls for a pair are deferred TWO pair
slots (not one) behind its scores/exp/EB, so the ScalarE exp and DVE EB
multiply get two pair-periods of slack before the PE consumes the attn
tile — launch-measured ablations showed the exp/EB chain, not PE cycles
alone, was exposed in the wall time.  With that slack, exp is a single
activation and PSUM-evacuation copies lean toward ScalarE (qk 10/12,
v 3/8, proj 3/6) to unload the busier VectorE.  EB multiplies stay on
DVE except two pairs per block on GPSIMD (Pool elementwise is ~4x
slower per element and its SBUF port contends with DVE, so larger
GPSIMD offload regresses; so does any PSUM re-packing — PE row-group
quadrants are hardwired to PSUM banks, and long-lived shared PSUM
tiles serialize under Tile dependency tracking).  Picked over the
qk-8/12 split (v9) on 3-of-4 paired HW comparisons, ~324-334 us/rep
vs ~370 us for v3.
"""

import sys

sys.path.insert(0, "/opt/trn_rl_repo")

import numpy as np
import ml_dtypes

import concourse.bacc as bacc
import concourse.mybir as mybir
import concourse.tile as tile
from concourse.bass_utils import run_bass_kernel_spmd

BF16 = ml_dtypes.bfloat16
F32 = np.float32

N_CORES = 8
D, H, HD = 384, 12, 32
WN = 49                      # tokens per window
NW = 64                      # distinct window masks
B_ = 2048
B_CORE = B_ // N_CORES       # 256 windows per core
T_CORE = B_CORE * WN         # 12544 tokens per core
PT = 2 * WN                  # 98 tokens per window pair
N_PAIR = B_CORE // 2         # 128 pairs per core
PAIR_PAT = NW // 2           # 32 distinct pair mask patterns
BLK_PAIRS = 8
BLK_T = BLK_PAIRS * PT       # 784 tokens per block
N_BLK = N_PAIR // BLK_PAIRS  # 16 blocks per core
NH = BLK_T // 2              # 392: half-block free dim for 512-limit psum
SCALE = HD ** (-0.5)

_BF = mybir.dt.bfloat16
_F32 = mybir.dt.float32

# which pairs' EB-multiply runs on GPSIMD (rest on DVE)
GPS_EB_PAIRS = frozenset({2, 5})


def _relative_position_index():
    coords = np.stack(np.meshgrid(np.arange(7), np.arange(7), indexing="ij"))
    cf = coords.reshape(2, -1)
    rel = cf[:, :, None] - cf[:, None, :]
    rel = rel.transpose(1, 2, 0).copy()
    rel[:, :, 0] += 6
    rel[:, :, 1] += 6
    rel[:, :, 0] *= 13
    return rel.sum(-1)  # [49, 49] int


def _chunk_range(p8: int, n: int):
    return range(p8 * n // BLK_PAIRS, (p8 + 1) * n // BLK_PAIRS)


def _build_nc(qkv_bias_nonzero: bool, reps: int = 1, skip: frozenset = frozenset()):
    nc = bacc.Bacc("TRN2", target_bir_lowering=False, debug=True)

    xT_d = nc.dram_tensor("xT", [D, T_CORE], _BF, kind="ExternalInput")
    wqk_d = nc.dram_tensor("wqk", [128, 3, 2 * D], _BF, kind="ExternalInput")
    wv_d = nc.dram_tensor("wv", [128, 3, D], _BF, kind="ExternalInput")
    pw_d = nc.dram_tensor("pw", [128, 3, D], _BF, kind="ExternalInput")
    eb_d = nc.dram_tensor("eb", [PT, PAIR_PAT, H, PT], _BF, kind="ExternalInput")
    bqk_d = nc.dram_tensor("bqk", [1, 2 * D], _BF, kind="ExternalInput")
    bv_d = nc.dram_tensor("bv", [1, D], _BF, kind="ExternalInput")
    yT_d = nc.dram_tensor("yT", [D, T_CORE], _BF, kind="ExternalOutput")

    xT_view = xT_d[:, :].rearrange("(k p) t -> p k t", p=128)
    yT_view = yT_d[:, :].rearrange("(k p) t -> p k t", p=128)

    TOT = N_BLK * reps

    with tile.TileContext(nc) as tc:
        with (
            tc.tile_pool(name="consts", bufs=1) as consts,
            tc.tile_pool(name="xin", bufs=2) as xin,
            tc.tile_pool(name="qkp", bufs=2) as qkp,
            tc.tile_pool(name="vp", bufs=2) as vp,
            tc.tile_pool(name="attnp", bufs=4) as attnp,
            tc.tile_pool(name="rp", bufs=2) as rp,
            tc.tile_pool(name="outp", bufs=3) as outp,
            tc.tile_pool(name="yp", bufs=2) as yp,
            tc.tile_pool(name="ps_mm", bufs=2, space="PSUM") as ps_mm,
            tc.tile_pool(name="ps_s", bufs=1, space="PSUM") as ps_s,
            tc.tile_pool(name="ps_cs", bufs=1, space="PSUM") as ps_cs,
            tc.tile_pool(name="ps_o", bufs=1, space="PSUM") as ps_o,
        ):
            # ---- constants ----
            wqk_sb = consts.tile([128, 3, 2 * D], _BF)
            nc.sync.dma_start(out=wqk_sb, in_=wqk_d[:, :, :])
            wv_sb = consts.tile([128, 3, D], _BF)
            nc.sync.dma_start(out=wv_sb, in_=wv_d[:, :, :])
            pw_sb = consts.tile([128, 3, D], _BF)
            nc.sync.dma_start(out=pw_sb, in_=pw_d[:, :, :])
            eb_sb = consts.tile([PT, PAIR_PAT, H, PT], _BF)
            nc.sync.dma_start(out=eb_sb, in_=eb_d[:, :, :, :])
            ones_sb = consts.tile([PT, 32], _BF)
            nc.vector.memset(ones_sb, 1.0)
            if "scores" in skip:
                attn0_sb = consts.tile([PT, H, PT], _BF, name="attn0")
                nc.gpsimd.memset(attn0_sb, 0.5)
            if "av" in skip:
                outN0_sb = consts.tile([128, 3, BLK_PAIRS, PT], _BF, name="outN0")
                nc.gpsimd.memset(outN0_sb, 0.5)
            if qkv_bias_nonzero:
                bqk_sb = consts.tile([1, 2 * D], _BF)
                nc.sync.dma_start(out=bqk_sb, in_=bqk_d[:, :])
                bv_sb = consts.tile([1, D], _BF)
                nc.sync.dma_start(out=bv_sb, in_=bv_d[:, :])
                onetok_sb = consts.tile([1, NH], _BF)
                nc.vector.memset(onetok_sb, 1.0)

            # ---- per-block helpers ----
            def dma_x(s, dst):
                t0 = (s % N_BLK) * BLK_T
                nc.sync.dma_start(out=dst, in_=xT_view[:, :, t0 : t0 + BLK_T])

            def qk_chunk(xT_sb, qk_sb, c):
                nh, m = c // 6, c % 6
                mm_ps = ps_mm.tile([128, NH], _F32, tag="mm")
                for k in range(3):
                    nc.tensor.matmul(
                        out=mm_ps,
                        lhsT=wqk_sb[:, k, 128 * m : 128 * (m + 1)],
                        rhs=xT_sb[:, k, nh * NH : (nh + 1) * NH],
                        start=(k == 0),
                        stop=(k == 2) if not qkv_bias_nonzero else False,
                    )
                if qkv_bias_nonzero:
                    nc.tensor.matmul(
                        out=mm_ps,
                        lhsT=bqk_sb[:, 128 * m : 128 * (m + 1)],
                        rhs=onetok_sb,
                        start=False,
                        stop=True,
                    )
                dst = qk_sb[:, m, nh * NH : (nh + 1) * NH]
                if c not in (5, 11):
                    nc.scalar.copy(out=dst, in_=mm_ps)
                else:
                    nc.vector.tensor_copy(out=dst, in_=mm_ps)

            def v_chunk(xT_sb, v_sb, p8):
                v_ps = ps_mm.tile([PT, D], _F32, tag="mm", name="v_ps")
                for k in range(3):
                    nc.tensor.matmul(
                        out=v_ps,
                        lhsT=xT_sb[:, k, p8 * PT : (p8 + 1) * PT],
                        rhs=wv_sb[:, k, :],
                        start=(k == 0),
                        stop=(k == 2) if not qkv_bias_nonzero else False,
                    )
                if qkv_bias_nonzero:
                    nc.tensor.matmul(
                        out=v_ps,
                        lhsT=onetok_sb[:, :PT],
                        rhs=bv_sb,
                        start=False,
                        stop=True,
                    )
                if p8 % 3 == 0:
                    nc.scalar.copy(out=v_sb[:, p8, :], in_=v_ps)
                else:
                    nc.vector.tensor_copy(out=v_sb[:, p8, :], in_=v_ps)

            def scores_exp(qk_sb, blk, p8):
                """scores matmuls + exp + EB-mul for pair (blk, p8).
                Returns the attn SBUF tile."""
                pr = (blk * BLK_PAIRS + p8) % PAIR_PAT
                ts = p8 * PT
                if "scores" in skip:
                    return attn0_sb
                # one 4-bank PSUM tile; bank j <- row-group j only
                s_ps = ps_s.tile([PT, 16, 128], _F32, tag="s", name="s4")
                for r in range(3):
                    for j in range(4):
                        g = (j + r) % 3
                        nc.tensor.matmul(
                            out=s_ps[:, 4 * j + g, :PT],
                            lhsT=qk_sb[32 * j : 32 * (j + 1), 3 + g, ts : ts + PT],
                            rhs=qk_sb[32 * j : 32 * (j + 1), g, ts : ts + PT],
                            start=True,
                            stop=True,
                            tile_position=(32 * j, 0),
                        )
                # attn = exp(s) * EB; head order (j, g): idx = 3j+g = head 4g+j
                # single exp/EB: the 2-pair stagger provides the chain slack
                attn_sb = attnp.tile([PT, H, PT], _BF)
                s_v = s_ps.rearrange("p (j g) c -> p j g c", g=4)
                nc.scalar.activation(
                    out=attn_sb[:, :, :],
                    in_=s_v[:, :, :3, :PT],
                    func=mybir.ActivationFunctionType.Exp,
                )
                if "ebmul" not in skip:
                    if p8 in GPS_EB_PAIRS:
                        nc.gpsimd.tensor_mul(attn_sb, attn_sb, eb_sb[:, pr, :, :])
                    else:
                        nc.vector.tensor_mul(attn_sb, attn_sb, eb_sb[:, pr, :, :])
                return attn_sb

            def colsum_recip(attn_sb):
                cs_ps = ps_cs.tile([128, 3, PT], _F32, tag="cs")
                for r in range(4):
                    for g in range(3):
                        j = (g + r) % 4
                        nc.tensor.matmul(
                            out=cs_ps[32 * j : 32 * (j + 1), g, :],
                            lhsT=ones_sb,
                            rhs=attn_sb[:, 3 * j + g, :],
                            start=True,
                            stop=True,
                            tile_position=(0, 32 * j),
                        )
                r_sb = rp.tile([128, 3, PT], _F32, tag="rsb")
                nc.vector.reciprocal_approx_fast(out=r_sb, in_=cs_ps)
                return r_sb

            def av_norm(v_sb, attn_sb, r_sb, outN_sb, p8):
                o_ps = ps_o.tile([128, 3, PT], _F32, tag="o")
                for r in range(4):
                    for g in range(3):
                        j = (g + r) % 4
                        h = 4 * g + j
                        nc.tensor.matmul(
                            out=o_ps[32 * j : 32 * (j + 1), g, :],
                            lhsT=v_sb[:, p8, 32 * h : 32 * (h + 1)],
                            rhs=attn_sb[:, 3 * j + g, :],
                            start=True,
                            stop=True,
                            tile_position=(0, 32 * j),
                        )
                nc.vector.tensor_mul(
                    outN_sb[:, :, p8, :], o_ps[:, :, :], r_sb[:, :, :]
                )

            def proj_chunk(outN_sb, yT_sb, c):
                nh, m = c // 3, c % 3
                y_ps = ps_mm.tile([128, NH], _F32, tag="mm")
                for k in range(3):
                    nc.tensor.matmul(
                        out=y_ps,
                        lhsT=pw_sb[:, k, 128 * m : 128 * (m + 1)],
                        rhs=outN_sb[:, k, nh * 4 : (nh + 1) * 4, :],
                        start=(k == 0),
                        stop=(k == 2),
                    )
                dst = yT_sb[:, m, nh * NH : (nh + 1) * NH]
                if c % 2 == 0:
                    nc.scalar.copy(out=dst, in_=y_ps)
                else:
                    nc.vector.tensor_copy(out=dst, in_=y_ps)

            # ---- software-pipelined main loop ----
            # step s: qk/v for block s, attention pairs for block s-1,
            # proj for block s-2, DMA x for s+1, DMA y for s-2.
            x_cur = None      # xT tile for block s
            qk_prev = qk_cur = None
            v_prev = v_cur = None
            outN_cur = outN_prev = None   # outN for block s-1 / s-2
            pend = []         # (attn_sb, v_sb, outN_sb, p8) awaiting colsum/av

            x_cur = xin.tile([128, 3, BLK_T], _BF)
            dma_x(0, x_cur)

            for s in range(TOT + 2):
                do_qkv = s < TOT
                do_attn = 0 <= s - 1 < TOT
                do_proj = 0 <= s - 2 < TOT

                qk_prev, qk_cur = qk_cur, (
                    qkp.tile([128, 6, BLK_T], _BF, name="qk_sb")
                    if do_qkv else None
                )
                v_prev, v_cur = v_cur, (
                    vp.tile([PT, BLK_PAIRS, D], _BF, name="v_sb")
                    if do_qkv else None
                )
                if do_qkv:
                    if s + 1 < TOT:
                        x_next = xin.tile([128, 3, BLK_T], _BF)
                        dma_x(s + 1, x_next)
                    else:
                        x_next = None
                outN_prev, outN_cur = outN_cur, (
                    (outN0_sb if "av" in skip
                     else outp.tile([128, 3, BLK_PAIRS, PT], _BF, name="outN_sb"))
                    if do_attn else None
                )
                if do_proj:
                    yT_sb = yp.tile([128, 3, BLK_T], _BF)

                for p8 in range(BLK_PAIRS):
                    if do_qkv:
                        for c in _chunk_range(p8, 12):
                            qk_chunk(x_cur, qk_cur, c)
                        v_chunk(x_cur, v_cur, p8)

                    if do_attn:
                        blk = (s - 1) % N_BLK
                        attn_sb = scores_exp(qk_prev, blk, p8)
                        if "av" not in skip:
                            pend.append((attn_sb, v_prev, outN_cur, p8))

                    # colsum/av + normalize for the pair TWO slots back, so
                    # exp+EBmul get two pair-periods of slack before the PE
                    # needs the attn tile.
                    if len(pend) > 2:
                        p_attn, p_v, p_outN, p_p8 = pend.pop(0)
                        r_sb = colsum_recip(p_attn)
                        av_norm(p_v, p_attn, r_sb, p_outN, p_p8)

                    if do_proj:
                        for c in _chunk_range(p8, 6):
                            proj_chunk(outN_prev, yT_sb, c)

                if do_qkv:
                    x_cur = x_next
                if do_proj:
                    t0 = ((s - 2) % N_BLK) * BLK_T
                    nc.sync.dma_start(
                        out=yT_view[:, :, t0 : t0 + BLK_T], in_=yT_sb
                    )

                # drain the pending pairs of the final attention block
                if s - 1 == TOT - 1:
                    while pend:
                        p_attn, p_v, p_outN, p_p8 = pend.pop(0)
                        r_sb = colsum_recip(p_attn)
                        av_norm(p_v, p_attn, r_sb, p_outN, p_p8)

    nc.compile()
    return nc


_NC_CACHE: dict = {}


def _get_nc(qkv_bias_nonzero: bool):
    key = qkv_bias_nonzero
    if key not in _NC_CACHE:
        _NC_CACHE[key] = _build_nc(qkv_bias_nonzero)
    return _NC_CACHE[key]


def _host_prep(x, mask, qkv_w, qkv_b, proj_w, rpb_table):
    """Build per-core input maps (all device tensors bf16)."""
    # x^T per core: [384, 12544], channel-major
    x8 = np.ascontiguousarray(x, dtype=F32).reshape(N_CORES, T_CORE, D)

    # weights: lhsT layout [ci, co] chunked as [128, 3, co]
    wqkv_t = np.ascontiguousarray(qkv_w, dtype=F32).T  # [384, 1152] = [ci, co]
    wqk = wqkv_t[:, : 2 * D].copy()
    wqk[:, :D] *= SCALE  # fold 1/sqrt(hd) into q weights
    wv = wqkv_t[:, 2 * D :]
    pw_t = np.ascontiguousarray(proj_w, dtype=F32).T  # [ci, co]

    def chunk(w):  # [384, co] -> [128, 3, co]
        return np.ascontiguousarray(
            w.reshape(3, 128, w.shape[1]).transpose(1, 0, 2)
        ).astype(BF16)

    wqk_a, wv_a, pw_a = chunk(wqk), chunk(wv), chunk(pw_t)

    # EB = exp(biasT + maskT) per pair pattern, [98, 32, 12, 98] (j, pr, h, i)
    rpi = _relative_position_index()
    bias = np.asarray(rpb_table, dtype=F32)[rpi]          # [i, j, H]
    biasT = bias.transpose(2, 1, 0)                        # [H, j, i]
    maskT = np.asarray(mask, dtype=F32).transpose(0, 2, 1)  # [w, j, i]
    mb = np.full((PAIR_PAT, H, PT, PT), -30000.0, dtype=F32)
    mb[:, :, :WN, :WN] = biasT[None] + maskT[0::2, None, :, :]
    mb[:, :, WN:, WN:] = biasT[None] + maskT[1::2, None, :, :]
    eb = np.exp(mb)
    # device head order is (j, g): idx = 3j + g holds head h = 4g + j
    perm = np.array([4 * (i % 3) + i // 3 for i in range(H)])
    eb = eb[:, perm]
    eb_a = np.ascontiguousarray(eb.transpose(2, 0, 1, 3)).astype(BF16)

    b = np.asarray(qkv_b, dtype=F32)
    bqk = b[: 2 * D].copy()
    bqk[:D] *= SCALE
    bqk_a = bqk[None, :].astype(BF16)
    bv_a = b[2 * D :][None, :].astype(BF16)

    in_maps = []
    for c in range(N_CORES):
        xT_c = np.ascontiguousarray(x8[c].T).astype(BF16)  # [384, 12544]
        in_maps.append(
            {
                "xT": xT_c,
                "wqk": wqk_a,
                "wv": wv_a,
                "pw": pw_a,
                "eb": eb_a,
                "bqk": bqk_a,
                "bv": bv_a,
            }
        )
    return in_maps


def kernel(x, mask, qkv_w, qkv_b, proj_w, rpb_table, _want_trace=False):
    qkv_bias_nonzero = bool(np.any(np.asarray(qkv_b) != 0))
    nc = _get_nc(qkv_bias_nonzero)
    in_maps = _host_prep(x, mask, qkv_w, qkv_b, proj_w, rpb_table)
    res = run_bass_kernel_spmd(
        nc, in_maps, core_ids=list(range(N_CORES))
    )
    yT = np.stack([res.results[c]["yT"].astype(F32) for c in range(N_CORES)])
    y = yT.transpose(0, 2, 1).reshape(B_, WN, D)
    return y

